# revision 23
# baseline (speedup 1.0000x reference)
"""AZConv2d Trainium2 kernel — W-major banded-matmul stencil design.

Math (per batch, from the reference):
  mu = softmax_r(gate_w @ x + gate_b)                      [4, L]
  alpha[r,s,l] = mu[r,l] * mu[r,l+d_s] * kern[r,s]
  agg[(r,c),l] = sum_s alpha[r,s,l]/asum[l] * x[c,l+d_s]
  out = pw_w @ agg + pw_b

Identity: with mu = E/Z the center 1/Z cancels between numerator and
normalizer:
  out[o,l] = sum_r ehat[r,l] * (pw_r @ conv3x3(mu_r*x, kern_r))[o,l]
  ehat = E / sum_r E_r * conv3x3(mu_r)            (Z-free)

Layouts:
  * W-major [w=128 interior cols on partitions, free=(c, h)]: the 3x3
    conv becomes 3 accumulating matmuls with tridiagonal band weights
    (dx mixing via the 128x128 band, dy via +-1 free offsets). All
    per-pixel normalization tensors are [128, 66] tiles, and the
    nu-broadcast multiply runs in the DVE 4x perf mode.
  * Gate (1x1 conv) runs C-major; exp(+bias) on the Act engine writes
    E (interior w) which one XBAR DMA-transpose converts to W-major.
  * conv PSUM is scaled by ehat into D[w,(h,c)] (bf16, free-dim reorder
    via strided APs), and an XBAR DMA-transpose of the interior rows
    yields T[((h-1)%2,c), ((h-1)//2, w)] whose 64-partition parity
    slices feed the pointwise matmul; bias enters as a rank-1 matmul.
  * Output PSUM is staged through the Act engine; output DMAs go on the
    gpsimd SWDGE queue to keep the SP sequencer free.

Sharding: batch B=8 -> one batch per NeuronCore. Image processed in two
H-halves (66 padded rows: halo + 64 + halo).
"""

import numpy as np

import concourse.bass as bass
import concourse.bacc as bacc
import concourse.mybir as mybir
import concourse.tile as tile
from concourse.bass_utils import run_bass_kernel_spmd

# ---- problem constants (hardcoded per contract) ----
B, C, H, W = 8, 64, 128, 128
R, COUT = 4, 128
PW_, PH_ = 130, 130            # padded image
HB = 66                        # padded rows per half (1 halo + 64 + 1 halo)
GL = HB * PW_                  # 8580 gate pixels per half (padded w)
FH = C * HB                    # 4224 = free size of a W-major half (c, h)
NM = 8                         # stencil/nu margin
GCH = 260                      # gate chunk = 2 padded rows
NG = GL // GCH                 # 33
SC = 7                         # channels per stencil chunk
SCH = SC * HB                  # 462
NST = 10                       # 9 chunks of 7c + 1 of 1c
PWB = 4                        # output rows per pw chunk
NPW = 64 // PWB                # 16

BF = mybir.dt.bfloat16
F32 = mybir.dt.float32
F32R = mybir.dt.float32r
F16 = mybir.dt.float16

_CACHED = {}


def _build():
    nc = bacc.Bacc(None, target_bir_lowering=False)
    x_cm = nc.dram_tensor("x_cm", [C, PH_ * PW_], F16, kind="ExternalInput")
    x_wm0 = nc.dram_tensor("x_wm0", [W, FH], BF, kind="ExternalInput")
    x_wm1 = nc.dram_tensor("x_wm1", [W, FH], BF, kind="ExternalInput")
    gwh = nc.dram_tensor("gwh", [C, R], F16, kind="ExternalInput")
    gb = nc.dram_tensor("gb", [R, 1], F32, kind="ExternalInput")
    bands = nc.dram_tensor("bands", [W, 12 * W], BF, kind="ExternalInput")
    pwt = nc.dram_tensor("pwt", [W, 2 * COUT], BF, kind="ExternalInput")
    pwb = nc.dram_tensor("pwb", [COUT, 1], F32, kind="ExternalInput")
    y = nc.dram_tensor("y", [COUT, H * W], BF, kind="ExternalOutput")

    with tile.TileContext(nc) as tc:
        with (
            tc.tile_pool(name="consts", bufs=1) as consts,
            tc.tile_pool(name="ld", bufs=2) as ldp,
            tc.tile_pool(name="fat", bufs=1) as fat,
            tc.tile_pool(name="maps", bufs=2) as maps,
            tc.tile_pool(name="outc", bufs=8) as outp,
            tc.tile_pool(name="ps_s", bufs=2, space="PSUM") as ps_gp,
            tc.tile_pool(name="ps_st", bufs=4, space="PSUM") as ps_stp,
            tc.tile_pool(name="ps_pw", bufs=2, space="PSUM") as ps_pwp,
        ):
            c_gwh = consts.tile([C, R], F16, name="c_gwh")
            c_gb = consts.tile([R, 1], F32, name="c_gb")
            c_bands = consts.tile([W, 12 * W], BF, name="c_bands")
            c_pwt = consts.tile([W, 2 * COUT], BF, name="c_pwt")
            c_pwb = consts.tile([COUT, 1], F32, name="c_pwb")
            for t, d in [
                (c_gwh, gwh), (c_gb, gb), (c_bands, bands),
                (c_pwt, pwt), (c_pwb, pwb),
            ]:
                nc.sync.dma_start(out=t, in_=d[:, :])

            mul = mybir.AluOpType.mult
            add = mybir.AluOpType.add
            Exp = mybir.ActivationFunctionType.Exp

            # ---- per-half tiles (double-buffered where both halves live) ----
            XCM, XWM, E16, EW, NU, EHAT, EV = {}, {}, {}, {}, {}, {}, {}
            SM = {}
            for half in range(2):
                XCM[half] = ldp.tile([C, GL], F16, name="XCM")
                XWM[half] = ldp.tile([W, FH], BF, name="XWM")
                E16[half] = ldp.tile([16, HB * W], BF, name="E16")
                EW[half] = maps.tile([W, HB * 16], BF, name="EW")
                NU[half] = maps.tile([W, NM + R * HB + NM], BF, name="NU")
                EHAT[half] = maps.tile([W, R * HB], BF, name="EHAT")
                EV[half] = maps.tile([W, R * HB], BF, name="EV")
                SM[half] = [maps.tile([W, HB], BF, name=f"sm{i}")
                            for i in range(5)]

            def esl(half, r):
                t = EW[half]
                return bass.AP(tensor=t.tensor, offset=t.offset + r,
                               ap=[list(t.ap[0]), [16, HB]])

            # ---- loads (XCM in pieces so the gate starts early) ----
            NPC = 3
            pc = GL // NPC                       # 2860 = 22 padded rows
            for half in range(2):
                h0 = half * 64
                for k in range(NPC):
                    nc.sync.dma_start(
                        out=XCM[half][:, k * pc:(k + 1) * pc],
                        in_=bass.AP(tensor=x_cm, offset=h0 * PW_ + k * pc,
                                    ap=[[PH_ * PW_, C], [1, pc]]))
            for half in range(2):
                nc.sync.dma_start(
                    out=XWM[half],
                    in_=(x_wm0 if half == 0 else x_wm1)[:, :])

            # ---- both gates up front, in two h-sections each ----
            # (g0, ng, row0, nrows): gate chunks / E rows per section
            SECS = [(0, 17, 0, 34), (17, 16, 34, 32)]
            VSECS = [(0, 33), (33, 33)]
            for half in range(2):
                E16v = E16[half].rearrange("p (h w) -> p h w", w=W)
                for g0, ng, r0, nr in SECS:
                    for g in range(g0, g0 + ng):
                        ga = g * GCH
                        ps = ps_gp.tile([R, GCH], F32, name="ps_g", tag="g")
                        nc.tensor.matmul(ps, c_gwh, XCM[half][:, ga:ga + GCH],
                                         start=True, stop=True)
                        psv = ps.rearrange("p (h w) -> p h w", w=PW_)
                        nc.scalar.activation(
                            out=E16v[0:R, 2 * g:2 * g + 2, :],
                            in_=psv[:, :, 1:129],
                            func=Exp, bias=c_gb, scale=1.0)
                    # E -> W-major piece: EW[w, (h, j)] = E16[j, 128h + w]
                    nc.scalar.dma_start_transpose(
                        out=EW[half].rearrange(
                            "p (b j) -> p b j", j=16)[:, r0:r0 + nr, :],
                        in_=E16[half][:, r0 * W:(r0 + nr) * W])

            def esls(half, r, r0, nr):
                t = EW[half]
                return bass.AP(tensor=t.tensor,
                               offset=t.offset + r + 16 * r0,
                               ap=[list(t.ap[0]), [16, nr]])

            XRS = [fat.tile([W, NM + FH + NM], BF, name=f"XR{r}")
                   for r in range(R)]

            def emit_norm(half):
                """Z, nu per section (DVE small ops)."""
                nu = NU[half]
                za, zb, zs, zi, asi = SM[half]
                for _, _, r0, nr in SECS:
                    e = lambda r: esls(half, r, r0, nr)
                    zav, zbv = za[:, r0:r0 + nr], zb[:, r0:r0 + nr]
                    zsv, ziv = zs[:, r0:r0 + nr], zi[:, r0:r0 + nr]
                    nc.vector.tensor_tensor(out=zav, in0=e(0), in1=e(1),
                                            op=add)
                    nc.vector.tensor_tensor(out=zbv, in0=e(2), in1=e(3),
                                            op=add)
                    nc.vector.tensor_tensor(out=zsv, in0=zav, in1=zbv,
                                            op=add)
                    with nc.allow_low_precision(reason="bf16 pipeline"):
                        nc.vector.reciprocal(ziv, zsv)
                    for r in range(R):
                        nc.vector.tensor_tensor(
                            out=nu[:, NM + r * HB + r0:
                                   NM + r * HB + r0 + nr],
                            in0=e(r), in1=ziv, op=mul)
                    pad_h = 0 if half == 0 else HB - 1
                    if r0 <= pad_h < r0 + nr:
                        nc.vector.memset(
                            bass.AP(tensor=nu.tensor,
                                    offset=nu.offset + NM + pad_h,
                                    ap=[list(nu.ap[0]), [HB, R]]), 0.0)

            def emit_xr(half, secs):
                """X_r rows = x * nu_r (c-broadcast, 4x mode)."""
                nu = NU[half]
                for _, _, r0, nr in secs:
                    for r in range(R):
                        nc.vector.tensor_tensor(
                            out=bass.AP(
                                tensor=XRS[r].tensor,
                                offset=XRS[r].offset + NM + r0,
                                ap=[list(XRS[r].ap[0]), [HB, C], [1, nr]]),
                            in0=bass.AP(tensor=XWM[half].tensor,
                                        offset=XWM[half].offset + r0,
                                        ap=[list(XWM[half].ap[0]), [HB, C],
                                            [1, nr]]),
                            in1=bass.AP(tensor=nu.tensor,
                                        offset=nu.offset + NM + r * HB + r0,
                                        ap=[list(nu.ap[0]), [0, C],
                                            [1, nr]]),
                            op=mul)

            def emit_vehat(half):
                """V = conv3x3(nu); AS = sum_r E_r*V_r; ehat = E/AS."""
                nu, ehat, ev = NU[half], EHAT[half], EV[half]
                za, zb, zs, zi, asi = SM[half]
                for r0, nr in VSECS:
                    e = lambda r: esls(half, r, r0, nr)
                    for r in range(R):
                        ps_v = ps_gp.tile([W, nr], F32, name="ps_v", tag="g")
                        for j, dy in enumerate((-1, 0, 1)):
                            bsl = c_bands[:, (3 * r + j) * W:
                                          (3 * r + j + 1) * W]
                            nc.tensor.matmul(
                                ps_v, bsl,
                                nu[:, NM + r * HB + r0 + dy:
                                   NM + r * HB + r0 + nr + dy],
                                start=(j == 0), stop=(j == 2))
                        nc.vector.tensor_tensor(
                            out=ev[:, r * HB + r0:r * HB + r0 + nr],
                            in0=e(r), in1=ps_v, op=mul)
                    evs = lambda r: ev[:, r * HB + r0:r * HB + r0 + nr]
                    zav, zbv = za[:, r0:r0 + nr], zb[:, r0:r0 + nr]
                    zsv, asv = zs[:, r0:r0 + nr], asi[:, r0:r0 + nr]
                    nc.vector.tensor_tensor(out=zav, in0=evs(0), in1=evs(1),
                                            op=add)
                    nc.vector.tensor_tensor(out=zbv, in0=evs(2), in1=evs(3),
                                            op=add)
                    nc.vector.tensor_tensor(out=zsv, in0=zav, in1=zbv,
                                            op=add)
                    with nc.allow_low_precision(reason="bf16 pipeline"):
                        nc.vector.reciprocal(asv, zsv)
                    for r in range(R):
                        nc.vector.tensor_tensor(
                            out=ehat[:, r * HB + r0:r * HB + r0 + nr],
                            in0=e(r), in1=asv, op=mul)

            # XR margins zeroed once (shared tiles, stable zeros)
            for r in range(R):
                nc.vector.memset(XRS[r][:, 0:NM], 0.0)
                nc.vector.memset(XRS[r][:, NM + FH:], 0.0)
            # half0 norm/XR up front; half1 norm+V early, XR deferred to
            # its stencil block (XR tiles shared across halves)
            emit_norm(0)
            emit_xr(0, SECS)
            emit_vehat(0)
            emit_norm(1)
            emit_vehat(1)

            # stencil h-windows covering interior h 1..64
            WINS = [(1 + 7 * i, 7) for i in range(9)] + [(64, 1)]
            # transpose pieces: after window wi, rows [hs, hs+nr) are done
            PIECES = {4: (1, 32), 6: (33, 16), 7: (49, 8), 9: (57, 8)}
            PWEMIT = {6: range(0, 8), 8: range(8, 12)}

            # ---- per half: XR, stencil, D, transposes, pw, output ----
            for half in range(2):
                h0 = half * 64
                nu, ehat = NU[half], EHAT[half]
                XR = XRS
                if half == 1:
                    emit_xr(1, SECS)
                D = [fat.tile([W, HB * 2 * C], BF, name=f"D{p}")
                     for p in range(2)]
                T = [fat.tile([W, 64 * W], BF, name=f"T{p}")
                     for p in range(2)]

                # stencil + D (h-window chunks), piece-wise transposes,
                # pw chunks interleaved to keep PE hot
                def pw_chunk(ci):
                    b0 = ci * PWB
                    fln = PWB * W
                    ps_y = ps_pwp.tile([COUT, fln], F32, name="ps_y", tag="y")
                    for p in range(2):
                        nc.tensor.matmul(
                            ps_y,
                            c_pwt[:, p * COUT:(p + 1) * COUT],
                            T[p][:, b0 * W:b0 * W + fln],
                            start=(p == 0), stop=(p == 1))
                    oc = outp.tile([COUT, fln], BF, name="oc")
                    nc.scalar.activation(
                        out=oc, in_=ps_y,
                        func=mybir.ActivationFunctionType.Identity,
                        bias=c_pwb, scale=1.0)
                    nc.gpsimd.dma_start(
                        out=bass.AP(tensor=y, offset=(h0 + b0) * W,
                                    ap=[[H * W, COUT], [1, fln]]),
                        in_=oc)

                for p in range(2):
                    for wi, (hw0, wl) in enumerate(WINS):
                        ln = C * wl
                        for r in (2 * p, 2 * p + 1):
                            co = (r % 2) * C
                            ps_c = ps_stp.tile([W, ln], F32, name="ps_c",
                                               tag="s")
                            for j, dy in enumerate((-1, 0, 1)):
                                bsl = c_bands[:, (3 * r + j) * W:
                                              (3 * r + j + 1) * W]
                                nc.tensor.matmul(
                                    ps_c, bsl,
                                    bass.AP(tensor=XR[r].tensor,
                                            offset=XR[r].offset + NM + hw0
                                            + dy,
                                            ap=[list(XR[r].ap[0]), [HB, C],
                                                [1, wl]]),
                                    start=(j == 0), stop=(j == 2))
                            nc.vector.tensor_tensor(
                                out=bass.AP(tensor=D[p].tensor,
                                            offset=D[p].offset
                                            + hw0 * 2 * C + co,
                                            ap=[list(D[p].ap[0]), [1, C],
                                                [2 * C, wl]]),
                                in0=bass.AP(tensor=ps_c.tensor,
                                            offset=ps_c.offset,
                                            ap=[list(ps_c.ap[0]), [wl, C],
                                                [1, wl]]),
                                in1=bass.AP(tensor=ehat.tensor,
                                            offset=ehat.offset + r * HB
                                            + hw0,
                                            ap=[list(ehat.ap[0]), [0, C],
                                                [1, wl]]),
                                op=mul)
                        if wi in PIECES:
                            hs, nr = PIECES[wi]
                            nc.sync.dma_start_transpose(
                                out=T[p].rearrange(
                                    "q (b w) -> q b w",
                                    w=W)[:, hs - 1:hs - 1 + nr, :],
                                in_=D[p][:, hs * 2 * C:(hs + nr) * 2 * C])
                        if p == 1 and wi in PWEMIT:
                            for ci in PWEMIT[wi]:
                                pw_chunk(ci)
                for ci in range(12, NPW):
                    pw_chunk(ci)

    nc.compile()
    return nc


def _host_prep(inputs):
    import ml_dtypes
    x = np.asarray(inputs["x"], np.float32)
    gate_w = np.asarray(inputs["gate_w"], np.float32)
    gate_b = np.asarray(inputs["gate_b"], np.float32)
    theta = np.asarray(inputs["theta"], np.float32)
    rsu = np.asarray(inputs["raw_sigma_u"], np.float32)
    rss = np.asarray(inputs["raw_sigma_s"], np.float32)
    pw_w = np.asarray(inputs["pw_w"], np.float32)
    pw_b = np.asarray(inputs["pw_b"], np.float32)

    tobf = lambda a: np.ascontiguousarray(a, np.float32).astype(
        ml_dtypes.bfloat16)

    grid = np.arange(3, dtype=np.float32) - 1.0
    dy = np.repeat(grid, 3)
    dx = np.tile(grid, 3)
    ct, st = np.cos(theta)[:, None], np.sin(theta)[:, None]
    pu = ct * dx[None, :] + st * dy[None, :]
    ps = -st * dx[None, :] + ct * dy[None, :]
    su = (np.log1p(np.exp(rsu)) + 1e-4)[:, None]
    ss = (np.log1p(np.exp(rss)) + 1e-4)[:, None]
    kern = np.exp(-pu ** 2 / su ** 2 - ps ** 2 / ss ** 2)  # [R, 9]

    # C-major padded x (for gate)
    xp = np.zeros((B, C, PH_, PW_), np.float32)
    xp[:, :, 1:129, 1:129] = x
    xp = xp.reshape(B, C, PH_ * PW_)
    # W-major x per half: x_wm[w, c, j] = x[c, h0+j-1, w] (0 at pad rows)
    xwf = np.zeros((B, W, C, PH_), np.float32)
    xwf[:, :, :, 1:129] = x.transpose(0, 3, 1, 2)
    xw0 = np.ascontiguousarray(xwf[:, :, :, 0:HB]).reshape(B, W, FH)
    xw1 = np.ascontiguousarray(xwf[:, :, :, 64:64 + HB]).reshape(B, W, FH)

    # band matrices: bands[(r,dy)][w', w] = kern[r, (dy+1)*3 + (dx+1)],
    # dx = w' - w
    bands = np.zeros((W, 12 * W), np.float32)
    for r in range(R):
        for j in range(3):
            blk = np.zeros((W, W), np.float32)
            for dxi in (-1, 0, 1):
                v = kern[r, j * 3 + (dxi + 1)]
                for w in range(W):
                    wp = w + dxi
                    if 0 <= wp < W:
                        blk[wp, w] = v
            bands[:, (3 * r + j) * W:(3 * r + j + 1) * W] = blk

    pwt = np.zeros((W, 2 * COUT), np.float32)
    for p in range(2):
        pwt[0:C, p * COUT:(p + 1) * COUT] = pw_w[:, (2 * p) * C:
                                                 (2 * p + 1) * C].T
        pwt[C:2 * C, p * COUT:(p + 1) * COUT] = pw_w[:, (2 * p + 1) * C:
                                                     (2 * p + 2) * C].T

    def hilo(a):
        hi = np.asarray(a, np.float32).astype(ml_dtypes.bfloat16)
        lo = (np.asarray(a, np.float32)
              - hi.astype(np.float32)).astype(ml_dtypes.bfloat16)
        return hi, lo

    common = {
        "gwh": np.ascontiguousarray(gate_w.T).astype(np.float16),
        "gb": gate_b.reshape(R, 1).astype(np.float32),
        "bands": tobf(bands),
        "pwt": tobf(pwt),
        "pwb": pw_b.reshape(COUT, 1).astype(np.float32),
    }
    in_maps = []
    for b in range(B):
        m = dict(common)
        m["x_cm"] = xp[b].astype(np.float16)
        m["x_wm0"] = tobf(xw0[b])
        m["x_wm1"] = tobf(xw1[b])
        in_maps.append(m)
    return in_maps


def kernel(**inputs):
    if "nc" not in _CACHED:
        _CACHED["nc"] = _build()
    nc = _CACHED["nc"]
    in_maps = _host_prep(inputs)
    res = run_bass_kernel_spmd(nc, in_maps, core_ids=list(range(B)))
    out = np.stack([res.results[b]["y"].reshape(COUT, H, W)
                    for b in range(B)], axis=0)
    return out.astype(np.float32)


# revision 25
# speedup vs baseline: 1.0266x; 1.0266x over previous
"""AZConv2d Trainium2 kernel — W-major banded-matmul stencil design.

Math (per batch, from the reference):
  mu = softmax_r(gate_w @ x + gate_b)                      [4, L]
  alpha[r,s,l] = mu[r,l] * mu[r,l+d_s] * kern[r,s]
  agg[(r,c),l] = sum_s alpha[r,s,l]/asum[l] * x[c,l+d_s]
  out = pw_w @ agg + pw_b

Identity: with mu = E/Z the center 1/Z cancels between numerator and
normalizer:
  out[o,l] = sum_r ehat[r,l] * (pw_r @ conv3x3(mu_r*x, kern_r))[o,l]
  ehat = E / sum_r E_r * conv3x3(mu_r)            (Z-free)

Layouts:
  * W-major [w=128 interior cols on partitions, free=(c, h)]: the 3x3
    conv becomes 3 accumulating matmuls with tridiagonal band weights
    (dx mixing via the 128x128 band, dy via +-1 free offsets). All
    per-pixel normalization tensors are [128, 66] tiles, and the
    nu-broadcast multiply runs in the DVE 4x perf mode.
  * Gate (1x1 conv) runs C-major; exp(+bias) on the Act engine writes
    E (interior w) which one XBAR DMA-transpose converts to W-major.
  * conv PSUM is scaled by ehat into D[w,(h,c)] (bf16, free-dim reorder
    via strided APs), and an XBAR DMA-transpose of the interior rows
    yields T[((h-1)%2,c), ((h-1)//2, w)] whose 64-partition parity
    slices feed the pointwise matmul; bias enters as a rank-1 matmul.
  * Output PSUM is staged through the Act engine; output DMAs go on the
    gpsimd SWDGE queue to keep the SP sequencer free.

Sharding: batch B=8 -> one batch per NeuronCore. Image processed in two
H-halves (66 padded rows: halo + 64 + halo).
"""

import numpy as np

import concourse.bass as bass
import concourse.bacc as bacc
import concourse.mybir as mybir
import concourse.tile as tile
from concourse.bass_utils import run_bass_kernel_spmd

# ---- problem constants (hardcoded per contract) ----
B, C, H, W = 8, 64, 128, 128
R, COUT = 4, 128
PW_, PH_ = 130, 130            # padded image
HB = 66                        # padded rows per half (1 halo + 64 + 1 halo)
GL = HB * PW_                  # 8580 gate pixels per half (padded w)
FH = C * HB                    # 4224 = free size of a W-major half (c, h)
NM = 8                         # stencil/nu margin
GCH = 260                      # gate chunk = 2 padded rows
NG = GL // GCH                 # 33
SC = 7                         # channels per stencil chunk
SCH = SC * HB                  # 462
NST = 10                       # 9 chunks of 7c + 1 of 1c
PWB = 4                        # output rows per pw chunk
NPW = 64 // PWB                # 16

BF = mybir.dt.bfloat16
F32 = mybir.dt.float32
F32R = mybir.dt.float32r
F16 = mybir.dt.float16

_CACHED = {}


def _build():
    nc = bacc.Bacc(None, target_bir_lowering=False)
    x_cm = nc.dram_tensor("x_cm", [C, PH_ * PW_], F16, kind="ExternalInput")
    x_wm0 = nc.dram_tensor("x_wm0", [W, FH], BF, kind="ExternalInput")
    x_wm1 = nc.dram_tensor("x_wm1", [W, FH], BF, kind="ExternalInput")
    gwh = nc.dram_tensor("gwh", [C, R], F16, kind="ExternalInput")
    gb = nc.dram_tensor("gb", [R, 1], F32, kind="ExternalInput")
    bands = nc.dram_tensor("bands", [W, 12 * W], BF, kind="ExternalInput")
    pwt = nc.dram_tensor("pwt", [W, 2 * COUT], BF, kind="ExternalInput")
    pwb = nc.dram_tensor("pwb", [COUT, 1], F32, kind="ExternalInput")
    y = nc.dram_tensor("y", [COUT, H * W], BF, kind="ExternalOutput")

    with tile.TileContext(nc) as tc:
        with (
            tc.tile_pool(name="consts", bufs=1) as consts,
            tc.tile_pool(name="ld", bufs=2) as ldp,
            tc.tile_pool(name="fat", bufs=1) as fat,
            tc.tile_pool(name="maps", bufs=2) as maps,
            tc.tile_pool(name="outc", bufs=6) as outp,
            tc.tile_pool(name="ps_s", bufs=2, space="PSUM") as ps_gp,
            tc.tile_pool(name="ps_st", bufs=4, space="PSUM") as ps_stp,
            tc.tile_pool(name="ps_pw", bufs=2, space="PSUM") as ps_pwp,
        ):
            c_gwh = consts.tile([C, R], F16, name="c_gwh")
            c_gb = consts.tile([R, 1], F32, name="c_gb")
            c_bands = consts.tile([W, 12 * W], BF, name="c_bands")
            c_pwt = consts.tile([W, 2 * COUT], BF, name="c_pwt")
            c_pwb = consts.tile([COUT, 1], F32, name="c_pwb")
            for t, d in [
                (c_gwh, gwh), (c_gb, gb), (c_bands, bands),
                (c_pwt, pwt), (c_pwb, pwb),
            ]:
                nc.sync.dma_start(out=t, in_=d[:, :])

            mul = mybir.AluOpType.mult
            add = mybir.AluOpType.add
            Exp = mybir.ActivationFunctionType.Exp

            # ---- per-half tiles (double-buffered where both halves live) ----
            XCM, XWM, E16, EW, NU, EHAT, EV = {}, {}, {}, {}, {}, {}, {}
            SM = {}
            for half in range(2):
                XCM[half] = ldp.tile([C, GL], F16, name="XCM")
                XWM[half] = ldp.tile([W, FH], BF, name="XWM")
                E16[half] = ldp.tile([16, HB * W], BF, name="E16")
                EW[half] = maps.tile([W, HB * 16], BF, name="EW")
                NU[half] = maps.tile([W, NM + R * HB + NM], BF, name="NU")
                EHAT[half] = maps.tile([W, R * HB], BF, name="EHAT")
                EV[half] = maps.tile([W, R * HB], BF, name="EV")
                SM[half] = [maps.tile([W, HB], BF, name=f"sm{i}")
                            for i in range(5)]

            def esl(half, r):
                t = EW[half]
                return bass.AP(tensor=t.tensor, offset=t.offset + r,
                               ap=[list(t.ap[0]), [16, HB]])

            # ---- loads (XCM in pieces so the gate starts early) ----
            NPC = 3
            pc = GL // NPC                       # 2860 = 22 padded rows
            for half in range(2):
                h0 = half * 64
                for k in range(NPC):
                    nc.sync.dma_start(
                        out=XCM[half][:, k * pc:(k + 1) * pc],
                        in_=bass.AP(tensor=x_cm, offset=h0 * PW_ + k * pc,
                                    ap=[[PH_ * PW_, C], [1, pc]]))
            for half in range(2):
                nc.sync.dma_start(
                    out=XWM[half],
                    in_=(x_wm0 if half == 0 else x_wm1)[:, :])

            # ---- both gates up front, in two h-sections each ----
            # (g0, ng, row0, nrows): gate chunks / E rows per section
            SECS = [(0, 17, 0, 34), (17, 16, 34, 32)]
            VSECS = [(0, 33), (33, 33)]
            for half in range(2):
                E16v = E16[half].rearrange("p (h w) -> p h w", w=W)
                for g0, ng, r0, nr in SECS:
                    for g in range(g0, g0 + ng):
                        ga = g * GCH
                        ps = ps_gp.tile([R, GCH], F32, name="ps_g", tag="g")
                        nc.tensor.matmul(ps, c_gwh, XCM[half][:, ga:ga + GCH],
                                         start=True, stop=True)
                        psv = ps.rearrange("p (h w) -> p h w", w=PW_)
                        nc.scalar.activation(
                            out=E16v[0:R, 2 * g:2 * g + 2, :],
                            in_=psv[:, :, 1:129],
                            func=Exp, bias=c_gb, scale=1.0)
                    # E -> W-major piece: EW[w, (h, j)] = E16[j, 128h + w]
                    nc.scalar.dma_start_transpose(
                        out=EW[half].rearrange(
                            "p (b j) -> p b j", j=16)[:, r0:r0 + nr, :],
                        in_=E16[half][:, r0 * W:(r0 + nr) * W])

            def esls(half, r, r0, nr):
                t = EW[half]
                return bass.AP(tensor=t.tensor,
                               offset=t.offset + r + 16 * r0,
                               ap=[list(t.ap[0]), [16, nr]])

            XRS = [fat.tile([W, NM + FH + NM], BF, name=f"XR{r}")
                   for r in range(R)]

            def emit_norm(half):
                """Z, nu per section (DVE small ops)."""
                nu = NU[half]
                za, zb, zs, zi, asi = SM[half]
                for _, _, r0, nr in SECS:
                    e = lambda r: esls(half, r, r0, nr)
                    zav, zbv = za[:, r0:r0 + nr], zb[:, r0:r0 + nr]
                    zsv, ziv = zs[:, r0:r0 + nr], zi[:, r0:r0 + nr]
                    nc.vector.tensor_tensor(out=zav, in0=e(0), in1=e(1),
                                            op=add)
                    nc.vector.tensor_tensor(out=zbv, in0=e(2), in1=e(3),
                                            op=add)
                    nc.vector.tensor_tensor(out=zsv, in0=zav, in1=zbv,
                                            op=add)
                    with nc.allow_low_precision(reason="bf16 pipeline"):
                        nc.vector.reciprocal(ziv, zsv)
                    for r in range(R):
                        nc.vector.tensor_tensor(
                            out=nu[:, NM + r * HB + r0:
                                   NM + r * HB + r0 + nr],
                            in0=e(r), in1=ziv, op=mul)
                    pad_h = 0 if half == 0 else HB - 1
                    if r0 <= pad_h < r0 + nr:
                        nc.vector.memset(
                            bass.AP(tensor=nu.tensor,
                                    offset=nu.offset + NM + pad_h,
                                    ap=[list(nu.ap[0]), [HB, R]]), 0.0)

            def emit_xr(half, secs):
                """X_r rows = x * nu_r (c-broadcast, 4x mode)."""
                nu = NU[half]
                for _, _, r0, nr in secs:
                    for r in range(R):
                        nc.vector.tensor_tensor(
                            out=bass.AP(
                                tensor=XRS[r].tensor,
                                offset=XRS[r].offset + NM + r0,
                                ap=[list(XRS[r].ap[0]), [HB, C], [1, nr]]),
                            in0=bass.AP(tensor=XWM[half].tensor,
                                        offset=XWM[half].offset + r0,
                                        ap=[list(XWM[half].ap[0]), [HB, C],
                                            [1, nr]]),
                            in1=bass.AP(tensor=nu.tensor,
                                        offset=nu.offset + NM + r * HB + r0,
                                        ap=[list(nu.ap[0]), [0, C],
                                            [1, nr]]),
                            op=mul)

            def emit_vehat(half):
                """V = conv3x3(nu); AS = sum_r E_r*V_r; ehat = E/AS."""
                nu, ehat, ev = NU[half], EHAT[half], EV[half]
                za, zb, zs, zi, asi = SM[half]
                for r0, nr in VSECS:
                    e = lambda r: esls(half, r, r0, nr)
                    for r in range(R):
                        ps_v = ps_gp.tile([W, nr], F32, name="ps_v", tag="g")
                        for j, dy in enumerate((-1, 0, 1)):
                            bsl = c_bands[:, (3 * r + j) * W:
                                          (3 * r + j + 1) * W]
                            nc.tensor.matmul(
                                ps_v, bsl,
                                nu[:, NM + r * HB + r0 + dy:
                                   NM + r * HB + r0 + nr + dy],
                                start=(j == 0), stop=(j == 2))
                        nc.vector.tensor_tensor(
                            out=ev[:, r * HB + r0:r * HB + r0 + nr],
                            in0=e(r), in1=ps_v, op=mul)
                    evs = lambda r: ev[:, r * HB + r0:r * HB + r0 + nr]
                    zav, zbv = za[:, r0:r0 + nr], zb[:, r0:r0 + nr]
                    zsv, asv = zs[:, r0:r0 + nr], asi[:, r0:r0 + nr]
                    nc.vector.tensor_tensor(out=zav, in0=evs(0), in1=evs(1),
                                            op=add)
                    nc.vector.tensor_tensor(out=zbv, in0=evs(2), in1=evs(3),
                                            op=add)
                    nc.vector.tensor_tensor(out=zsv, in0=zav, in1=zbv,
                                            op=add)
                    with nc.allow_low_precision(reason="bf16 pipeline"):
                        nc.vector.reciprocal(asv, zsv)
                    for r in range(R):
                        nc.vector.tensor_tensor(
                            out=ehat[:, r * HB + r0:r * HB + r0 + nr],
                            in0=e(r), in1=asv, op=mul)

            # XR margins zeroed once (shared tiles, stable zeros)
            for r in range(R):
                nc.vector.memset(XRS[r][:, 0:NM], 0.0)
                nc.vector.memset(XRS[r][:, NM + FH:], 0.0)
            # half0 norm/XR up front; half1 norm+V early, XR deferred to
            # its stencil block (XR tiles shared across halves)
            emit_norm(0)
            emit_xr(0, SECS)
            emit_vehat(0)
            emit_norm(1)
            emit_vehat(1)

            # stencil h-windows covering interior h 1..64
            WINS = [(1 + 7 * i, 7) for i in range(9)] + [(64, 1)]
            # transpose pieces: after window wi, rows [hs, hs+nr) are done
            PIECES = {4: (1, 32), 6: (33, 16), 7: (49, 8), 9: (57, 8)}
            PWEMIT = {6: range(0, 8), 8: range(8, 12)}

            # ---- per half: XR, stencil, D, transposes, pw, output ----
            for half in range(2):
                h0 = half * 64
                nu, ehat = NU[half], EHAT[half]
                XR = XRS
                if half == 1:
                    emit_xr(1, SECS)
                D = [fat.tile([W, HB * 2 * C], BF, name=f"D{p}")
                     for p in range(2)]
                T = [fat.tile([W, 64 * W], BF, name=f"T{p}")
                     for p in range(2)]

                # stencil + D (h-window chunks), piece-wise transposes,
                # pw chunks interleaved to keep PE hot
                ocpair = {}

                def pw_chunk(ci):
                    b0 = ci * PWB
                    fln = PWB * W
                    ps_y = ps_pwp.tile([COUT, fln], F32, name="ps_y", tag="y")
                    for p in range(2):
                        nc.tensor.matmul(
                            ps_y,
                            c_pwt[:, p * COUT:(p + 1) * COUT],
                            T[p][:, b0 * W:b0 * W + fln],
                            start=(p == 0), stop=(p == 1))
                    if ci % 2 == 0:
                        ocpair[0] = outp.tile([COUT, 2 * fln], BF, name="oc")
                    oc = ocpair[0]
                    half_off = (ci % 2) * fln
                    if ci >= 8:
                        # late chunks: bias-add on the (idle) vector engine
                        nc.vector.tensor_scalar_add(
                            oc[:, half_off:half_off + fln], ps_y, c_pwb)
                    else:
                        nc.scalar.activation(
                            out=oc[:, half_off:half_off + fln], in_=ps_y,
                            func=mybir.ActivationFunctionType.Identity,
                            bias=c_pwb, scale=1.0)
                    if ci % 2 == 1:
                        # one merged DMA per chunk pair, alternating queues
                        eng = nc.sync if (ci // 2) % 2 == 0 else nc.gpsimd
                        eng.dma_start(
                            out=bass.AP(tensor=y,
                                        offset=(h0 + b0 - PWB) * W,
                                        ap=[[H * W, COUT], [1, 2 * fln]]),
                            in_=oc)

                for p in range(2):
                    for wi, (hw0, wl) in enumerate(WINS):
                        ln = C * wl
                        for r in (2 * p, 2 * p + 1):
                            co = (r % 2) * C
                            ps_c = ps_stp.tile([W, ln], F32, name="ps_c",
                                               tag="s")
                            for j, dy in enumerate((-1, 0, 1)):
                                bsl = c_bands[:, (3 * r + j) * W:
                                              (3 * r + j + 1) * W]
                                nc.tensor.matmul(
                                    ps_c, bsl,
                                    bass.AP(tensor=XR[r].tensor,
                                            offset=XR[r].offset + NM + hw0
                                            + dy,
                                            ap=[list(XR[r].ap[0]), [HB, C],
                                                [1, wl]]),
                                    start=(j == 0), stop=(j == 2))
                            nc.vector.tensor_tensor(
                                out=bass.AP(tensor=D[p].tensor,
                                            offset=D[p].offset
                                            + hw0 * 2 * C + co,
                                            ap=[list(D[p].ap[0]), [1, C],
                                                [2 * C, wl]]),
                                in0=bass.AP(tensor=ps_c.tensor,
                                            offset=ps_c.offset,
                                            ap=[list(ps_c.ap[0]), [wl, C],
                                                [1, wl]]),
                                in1=bass.AP(tensor=ehat.tensor,
                                            offset=ehat.offset + r * HB
                                            + hw0,
                                            ap=[list(ehat.ap[0]), [0, C],
                                                [1, wl]]),
                                op=mul)
                        if wi in PIECES:
                            hs, nr = PIECES[wi]
                            nc.sync.dma_start_transpose(
                                out=T[p].rearrange(
                                    "q (b w) -> q b w",
                                    w=W)[:, hs - 1:hs - 1 + nr, :],
                                in_=D[p][:, hs * 2 * C:(hs + nr) * 2 * C])
                        if p == 1 and wi in PWEMIT:
                            for ci in PWEMIT[wi]:
                                pw_chunk(ci)
                for ci in range(12, NPW):
                    pw_chunk(ci)

    nc.compile()
    return nc


def _host_prep(inputs):
    import ml_dtypes
    x = np.asarray(inputs["x"], np.float32)
    gate_w = np.asarray(inputs["gate_w"], np.float32)
    gate_b = np.asarray(inputs["gate_b"], np.float32)
    theta = np.asarray(inputs["theta"], np.float32)
    rsu = np.asarray(inputs["raw_sigma_u"], np.float32)
    rss = np.asarray(inputs["raw_sigma_s"], np.float32)
    pw_w = np.asarray(inputs["pw_w"], np.float32)
    pw_b = np.asarray(inputs["pw_b"], np.float32)

    tobf = lambda a: np.ascontiguousarray(a, np.float32).astype(
        ml_dtypes.bfloat16)

    grid = np.arange(3, dtype=np.float32) - 1.0
    dy = np.repeat(grid, 3)
    dx = np.tile(grid, 3)
    ct, st = np.cos(theta)[:, None], np.sin(theta)[:, None]
    pu = ct * dx[None, :] + st * dy[None, :]
    ps = -st * dx[None, :] + ct * dy[None, :]
    su = (np.log1p(np.exp(rsu)) + 1e-4)[:, None]
    ss = (np.log1p(np.exp(rss)) + 1e-4)[:, None]
    kern = np.exp(-pu ** 2 / su ** 2 - ps ** 2 / ss ** 2)  # [R, 9]

    # C-major padded x (for gate)
    xp = np.zeros((B, C, PH_, PW_), np.float32)
    xp[:, :, 1:129, 1:129] = x
    xp = xp.reshape(B, C, PH_ * PW_)
    # W-major x per half: x_wm[w, c, j] = x[c, h0+j-1, w] (0 at pad rows)
    xwf = np.zeros((B, W, C, PH_), np.float32)
    xwf[:, :, :, 1:129] = x.transpose(0, 3, 1, 2)
    xw0 = np.ascontiguousarray(xwf[:, :, :, 0:HB]).reshape(B, W, FH)
    xw1 = np.ascontiguousarray(xwf[:, :, :, 64:64 + HB]).reshape(B, W, FH)

    # band matrices: bands[(r,dy)][w', w] = kern[r, (dy+1)*3 + (dx+1)],
    # dx = w' - w
    bands = np.zeros((W, 12 * W), np.float32)
    for r in range(R):
        for j in range(3):
            blk = np.zeros((W, W), np.float32)
            for dxi in (-1, 0, 1):
                v = kern[r, j * 3 + (dxi + 1)]
                for w in range(W):
                    wp = w + dxi
                    if 0 <= wp < W:
                        blk[wp, w] = v
            bands[:, (3 * r + j) * W:(3 * r + j + 1) * W] = blk

    pwt = np.zeros((W, 2 * COUT), np.float32)
    for p in range(2):
        pwt[0:C, p * COUT:(p + 1) * COUT] = pw_w[:, (2 * p) * C:
                                                 (2 * p + 1) * C].T
        pwt[C:2 * C, p * COUT:(p + 1) * COUT] = pw_w[:, (2 * p + 1) * C:
                                                     (2 * p + 2) * C].T

    def hilo(a):
        hi = np.asarray(a, np.float32).astype(ml_dtypes.bfloat16)
        lo = (np.asarray(a, np.float32)
              - hi.astype(np.float32)).astype(ml_dtypes.bfloat16)
        return hi, lo

    common = {
        "gwh": np.ascontiguousarray(gate_w.T).astype(np.float16),
        "gb": gate_b.reshape(R, 1).astype(np.float32),
        "bands": tobf(bands),
        "pwt": tobf(pwt),
        "pwb": pw_b.reshape(COUT, 1).astype(np.float32),
    }
    in_maps = []
    for b in range(B):
        m = dict(common)
        m["x_cm"] = xp[b].astype(np.float16)
        m["x_wm0"] = tobf(xw0[b])
        m["x_wm1"] = tobf(xw1[b])
        in_maps.append(m)
    return in_maps


def kernel(**inputs):
    if "nc" not in _CACHED:
        _CACHED["nc"] = _build()
    nc = _CACHED["nc"]
    in_maps = _host_prep(inputs)
    res = run_bass_kernel_spmd(nc, in_maps, core_ids=list(range(B)))
    out = np.stack([res.results[b]["y"].reshape(COUT, H, W)
                    for b in range(B)], axis=0)
    return out.astype(np.float32)
